# revision 2
# baseline (speedup 1.0000x reference)
"""Additive (Bahdanau) attention on 8 Trainium2 NeuronCores — Fourier edition.

Replaces the O(Q*K*H) ACT tanh (the baseline bottleneck, ~34us/core) with a
separable Fourier expansion:  tanh(s) ~ sum_m b_m sin(om_m s), om_m =
(m-1/2)*pi/L, and sin(om(q+k)) = sin_q cos_k + cos_q sin_k.  Per-side
features sin/cos(om_m x) are built once per key/query element (2M passes
instead of Q), then the (j,k) score surface comes from PE matmuls with
contraction (h, mode):

  scores[j,k] = sum_m sum_h [b_m wv_h cos(om_m q_jh)] sin(om_m k_kh)
                          + [b_m wv_h sin(om_m q_jh)] cos(om_m k_kh)

Feature construction (the critical trick): ACT Sin is only valid on
[-pi,pi], so only the half-angle base chunk1 = [sin(om0/2 x); cos(om0/2 x)]
is computed directly (args <= 1.23 rad).  Higher modes come from the
Chebyshev 3-term recurrence on DVE in fp16 at 4x rate, in SCALED form
u~_m = u_m / 2^(m-1) so every intermediate stays in [-1,1] (fp16 ulp stays
relative; the naive recurrence's 2u_m-u_{m-1} intermediates at |.|~3 lose
2 mantissa bits per step and blow up 5e-2 by m=8).  The multiplier is kept
as dh = (w-2)/2 = -2 sin^2(om0 x / 2) (relative-precise near theta=0, where
storing w=2cos itself in fp16 destroys the phase), computed from the
u-half only (the v-half's 2cos^2-2 cancels catastrophically).  Steps:
  t = (dh + 1) * u~_m          [STT, fp32 internal]
  u~_{m+1} = -0.25 u~_{m-1} + t [STT]
The 4^(m-1) rescale folds into the q-side coefficient tile for free.
Measured end-to-end (HW-exact numpy model, real inputs): 3.6e-3 vs the
2e-2 gate, same as the baseline's bf16 error.

All pairs+batches+q-sides share ONE merged chain over a [128, F] column
space (F = 2*(T0+T1) + 4*64), so the recurrence is 2 DVE ops per mode
total.  The q-side lhsT tiles are built by 2 strided TT ops against a
precomputed coef tile (wv x b_m 4^(m-1), PE outer product during the DMA
window).  Input-specialized (T0,T1) classes, big-with-small pairing, DMA
ordering, additive -1e9 mask via PE, and the exp/attn@values epilogue are
inherited from the baseline kernel.
"""

import os
import sys

import numpy as np

for _p in ("/opt/trn_rl_repo", "/root/.axon_site/_ro/trn_rl_repo"):
    if os.path.isdir(_p) and _p not in sys.path:
        sys.path.append(_p)

B, Q, K, H, V = 32, 64, 512, 64, 64
NCORES = 8
BPC = B // NCORES          # batches per core
NPAIR = BPC // 2           # batch pairs per core
NEG = -1e9
MAXPROGS = int(os.environ.get("MAXPROGS", "8"))

M = int(os.environ.get("FMODES", "7"))      # Fourier modes
LDOM = float(os.environ.get("FL", "6.8"))   # half-domain for the fit

_NC_CACHE = {}
_COEF_CACHE = {}


def fourier_coefs(M=M, L=LDOM):
    key = (M, L)
    if key not in _COEF_CACHE:
        om = (np.arange(1, M + 1) - 0.5) * np.pi / L
        s = np.linspace(-L, L, 4001)
        w = np.exp(-s * s / 4.0) ** 0.3 + 0.02
        A = np.sin(np.outer(s, om))
        Wt = np.sqrt(w)[:, None]
        b, *_ = np.linalg.lstsq(A * Wt, np.tanh(s) * Wt[:, 0], rcond=None)
        _COEF_CACHE[key] = b.astype(np.float64)
    return _COEF_CACHE[key]


def build_nc(T0=512, T1=512, kdma_merge=True, mask_after=2, epi_after=None):
    import concourse.bass as bass  # noqa: F401
    from concourse import mybir
    from concourse import tile
    from concourse.masks import make_identity
    from concourse import bacc

    f32 = mybir.dt.float32
    bf16 = mybir.dt.bfloat16
    fp16 = mybir.dt.float16
    i32 = mybir.dt.int32
    Sin = mybir.ActivationFunctionType.Sin
    Exp = mybir.ActivationFunctionType.Exp
    Copy = mybir.ActivationFunctionType.Copy
    AOp = mybir.AluOpType

    om0 = np.pi / LDOM
    bcoef = fourier_coefs()

    nc = bacc.Bacc("TRN2", target_bir_lowering=False, debug=False, num_devices=NCORES)

    queries_d = nc.dram_tensor("queries", [BPC, Q, H], f32, kind="ExternalInput")
    keys_d = nc.dram_tensor("keys", [BPC, K, H], f32, kind="ExternalInput")
    values_d = nc.dram_tensor("values", [BPC, K, V], f32, kind="ExternalInput")
    vlens_d = nc.dram_tensor("valid_lens", [BPC, 1], i32, kind="ExternalInput")
    Wq_d = nc.dram_tensor("Wq", [H, H], f32, kind="ExternalInput")
    Wk_d = nc.dram_tensor("Wk", [H, H], f32, kind="ExternalInput")
    wv_d = nc.dram_tensor("wv", [H, 1], f32, kind="ExternalInput")
    # packed compile-time constants: cols 0:128 identity, 128:136 const cols
    # (bias_k, bias_q, pm15_k, pm15_q), 136:264 sel2 rows (rows 0:2)
    cdata_d = nc.dram_tensor("cdata", [128, 264], f32, kind="ExternalInput")
    out_d = nc.dram_tensor("out", [BPC, Q, V], f32, kind="ExternalOutput")
    DBG = os.environ.get("KDBG", "0") == "1"
    if DBG:
        dbg_d = nc.dram_tensor("dbg", [6, 128, 1536], f32, kind="ExternalOutput")

    Ts = [min(K, (int(t) + 7) // 8 * 8) for t in (T0, T1)]
    assert Ts[0] >= Ts[1] >= 8 and Ts[0] <= K
    CKs = [(t + 127) // 128 for t in Ts]    # 128-wide key chunks used
    # column layout of the merged feature space
    QCOLS = BPC * Q  # 256
    koff = [0, Ts[0], 2 * Ts[0], 2 * Ts[0] + Ts[1]]   # batch (p,b) -> col offset
    qoff = 2 * (Ts[0] + Ts[1])
    F = qoff + QCOLS

    if epi_after is None:
        epi_after = M  # pair0 epilogue issued after all its matmuls

    with tile.TileContext(nc) as tc:
        with (
            tc.tile_pool(name="consts", bufs=1) as consts,
            tc.tile_pool(name="work", bufs=2) as work,
            tc.tile_pool(name="soft", bufs=2) as soft,
            tc.tile_pool(name="ps", bufs=2, space="PSUM") as ps,
        ):
            # ---- input DMAs (priority order; see baseline) ----
            KMERGE = [kdma_merge and c >= 3 for c in CKs]
            KL = [K // 128 if m else c for c, m in zip(CKs, KMERGE)]
            keys_pair = []
            for p in range(NPAIR):
                kp = consts.tile([128, 2, KL[p], H], f32, name=f"keys_pair{p}")
                keys_pair.append(kp)

            def keys_dma(p):
                if KMERGE[p]:
                    nc.sync.dma_start(
                        out=keys_pair[p],
                        in_=keys_d[2 * p : 2 * p + 2].rearrange(
                            "b (c p) h -> p b c h", p=128
                        ),
                    )
                else:
                    for half in range(2):
                        nc.sync.dma_start(
                            out=keys_pair[p][:, half, :, :],
                            in_=keys_d[2 * p + half, 0 : CKs[p] * 128, :].rearrange(
                                "(c p) h -> p c h", p=128
                            ),
                        )

            keys_dma(0)
            cdata = consts.tile([128, 264], f32)
            nc.sync.dma_start(out=cdata, in_=cdata_d[:, :])
            Wk_sb = consts.tile([H, H], f32)
            nc.scalar.dma_start(out=Wk_sb, in_=Wk_d[:, :])
            q_all = consts.tile([Q, BPC, H], f32)
            nc.sync.dma_start(out=q_all, in_=queries_d.rearrange("b q h -> q b h"))
            Wq_sb = consts.tile([H, H], f32)
            nc.scalar.dma_start(out=Wq_sb, in_=Wq_d[:, :])
            wv_sb = consts.tile([H, 1], f32)
            nc.scalar.dma_start(out=wv_sb, in_=wv_d[:, :])
            vl_i = consts.tile([2, NPAIR, 1], i32)
            nc.scalar.dma_start(
                out=vl_i, in_=vlens_d.rearrange("(c r) o -> r c o", r=2)
            )
            keys_dma(1)

            # ---- values: off the critical path ----
            v0s = []
            for p in range(NPAIR):
                v0 = work.tile([128, 2, KL[p], V], f32, tag=f"v0_{p}", bufs=1)
                if KMERGE[p]:
                    nc.sync.dma_start(
                        out=v0,
                        in_=values_d[2 * p : 2 * p + 2].rearrange(
                            "b (c p) v -> p b c v", p=128
                        ),
                    )
                else:
                    for half in range(2):
                        nc.sync.dma_start(
                            out=v0[:, half, :, :],
                            in_=values_d[2 * p + half, 0 : CKs[p] * 128, :].rearrange(
                                "(c p) v -> p c v", p=128
                            ),
                        )
                v0s.append(v0)

            # ---- constants (ident on Pool: no DMA dependency, ready ~3us) ----
            ident0 = consts.tile([128, 128], f32)
            make_identity(nc, ident0)
            ident = consts.tile([128, 128], f32)
            nc.vector.tensor_copy(ident, ident0)

            # PE warmup + priming first: only needs ident
            prime_ps = ps.tile([32, 32], f32, tag="tail", bufs=2)
            nc.tensor.transpose(prime_ps, ident[0:32, 0:32], ident[0:32, 0:32])
            for _w in range(int(os.environ.get("WARMUP", "2"))):
                wps = ps.tile([128, 128], f32, tag="tail", bufs=2, name=f"wps{_w}")
                nc.tensor.transpose(wps, ident, ident)

            # dup weight lhsTs [c=64, p=128] = [W | W]
            Wq2 = consts.tile([H, 128], bf16)
            nc.vector.tensor_copy(Wq2[:, 0:64], Wq_sb)
            nc.vector.tensor_copy(Wq2[:, 64:128], Wq_sb)
            Wk2 = consts.tile([H, 128], bf16)
            nc.vector.tensor_copy(Wk2[:, 0:64], Wk_sb)
            nc.vector.tensor_copy(Wk2[:, 64:128], Wk_sb)

            # k-side chunk1 = [cos-half; sin-half] (v on top: the score matmul
            # pairs k rows with the UNswapped q-chain), q-side = [sin; cos];
            # per-partition bias / chunk2-scalar columns come in via cdata
            bias_k = cdata[:, 128:129]
            bias_q = cdata[:, 129:130]
            pm15_k = cdata[:, 130:131]
            pm15_q = cdata[:, 131:132]

            # small consts on Pool (keeps DVE clear; PE merges Pool waits)
            iota2_i = consts.tile([2, 512], i32)
            nc.gpsimd.iota(iota2_i, pattern=[[1, 512]], base=0, channel_multiplier=0)
            iota2 = consts.tile([2, 512], f32)
            nc.gpsimd.tensor_copy(iota2, iota2_i)
            vl_f = consts.tile([2, NPAIR, 1], f32)
            nc.gpsimd.tensor_copy(vl_f, vl_i)

            # mask row selector (fp16, pairs with amask)
            sel2 = consts.tile([2, 128], bf16)
            nc.vector.tensor_copy(sel2, cdata[0:2, 136:264])

            # coef row [1, M*QCOLS]: b_m * 4^(m-1), bf16 (Pool memsets)
            brow = consts.tile([1, M, QCOLS], bf16)
            for m in range(M):
                nc.gpsimd.memset(brow[:, m, :], float(bcoef[m] * (4.0 ** m)))
            # wv as [1, 128] row = [wv; wv] (PE transpose + copies)
            wvT_ps = ps.tile([1, 64], f32, tag="prep", bufs=2, name="wvT_ps")
            nc.tensor.transpose(wvT_ps, wv_sb, ident[0:64, 0:64])
            wvrow = consts.tile([1, 128], bf16)
            nc.vector.tensor_copy(wvrow[:, 0:64], wvT_ps)
            nc.vector.tensor_copy(wvrow[:, 64:128], wvT_ps)

            # first ACT compute op = tiny Sin: loads the trig table during
            # the DMA window
            sintab = consts.tile([1, 1], f32)
            nc.vector.memset(sintab, 0.0)
            sindum = consts.tile([1, 1], fp16)
            nc.scalar.activation(sindum, sintab, Sin)

            # wv duplicated as a [128,1] per-partition column (for lhsT TS)
            wv_col = consts.tile([128, 1], f32)
            nc.vector.tensor_copy(wv_col[0:64, :], wv_sb)
            nc.vector.tensor_copy(wv_col[64:128, :], wv_sb)

            # ---- per-pair masks (Pool; late) ----
            amask = [
                consts.tile([2, Ts[p]], bf16, name=f"amask{p}") for p in range(NPAIR)
            ]

            def late_mask(p):
                nc.gpsimd.tensor_scalar(
                    out=amask[p],
                    in0=iota2[:, 0 : Ts[p]],
                    scalar1=vl_f[:, p, :],
                    scalar2=NEG,
                    op0=AOp.is_ge,
                    op1=AOp.mult,
                )

            vals_pair = [
                consts.tile([128, 2, CKs[p], V], bf16, name=f"vals{p}")
                for p in range(NPAIR)
            ]

            def late_vals(p):
                src_v = v0s[p][:, :, 0 : CKs[p], :] if KL[p] > CKs[p] else v0s[p]
                nc.gpsimd.tensor_copy(vals_pair[p], src_v)

            # ---- prep: transposes + projections into PSUM ----
            # kW2[b] [128, T] f32 PSUM (rows duplicated), qW2all [128, 256]
            kw_ps = [None] * 4

            def prep_pair(p):
                T, CK = Ts[p], CKs[p]
                for half in range(2):
                    keysT_ps = ps.tile(
                        [H, 512], f32, tag="prep", bufs=2,
                        name=f"keysT_ps{2*p+half}",
                    )
                    for cc in range(CK):
                        nc.tensor.transpose(
                            keysT_ps[:, 128 * cc : 128 * (cc + 1)],
                            keys_pair[p][:, half, cc, :],
                            ident,
                        )
                    keysT_sb = work.tile([H, CK * 128], bf16, tag="keysT_sb", bufs=4)
                    nc.vector.tensor_copy(keysT_sb, keysT_ps[:, 0 : CK * 128])
                    kwp = ps.tile([128, T], f32, tag="big", bufs=4,
                                  name=f"kw_ps{2*p+half}")
                    nc.tensor.matmul(
                        kwp, lhsT=Wk2, rhs=keysT_sb[:, 0:T],
                        start=True, stop=True,
                    )
                    kw_ps[2 * p + half] = kwp

            qw_ps = ps.tile([128, QCOLS], f32, tag="tail", bufs=2, name="qw_ps")

            def prep_q():
                for b in range(BPC):
                    qT_ps = ps.tile([H, Q], f32, tag="prep", bufs=2, name=f"qT_ps{b}")
                    nc.tensor.transpose(qT_ps, q_all[:, b, :], ident[0:Q, 0:Q])
                    qT_sb = work.tile([H, Q], bf16, tag="qT_sb", bufs=4)
                    nc.vector.tensor_copy(qT_sb, qT_ps)
                    nc.tensor.matmul(
                        qw_ps[:, Q * b : Q * (b + 1)], lhsT=Wq2, rhs=qT_sb,
                        start=True, stop=True,
                    )

            prep_pair(0)
            prep_q()
            prep_pair(1)

            # ---- merged feature space: [p0k (2T0) | p1k (2T1) | p0q | p1q] ----
            F0k, F1k = 2 * Ts[0], 2 * Ts[1]
            QB = 2 * Q  # q cols per pair
            off_p1k = F0k
            off_q = [F0k + F1k, F0k + F1k + QB]
            F = F0k + F1k + 2 * QB

            s2_k = cdata[:, 132:133]   # [7.5pi]*64 + [7pi]*64   (cos rows on top)
            s2_q = cdata[:, 133:134]   # [7pi]*64 + [7.5pi]*64   (sin rows on top)

            feats = [None] * (M + 1)

            def kslice(p, half):
                base = off_p1k * p
                return slice(base + half * Ts[p], base + (half + 1) * Ts[p])

            # mode-1 features: direct ACT Sin from the f32 PSUM projections
            def mode1_pair(p):
                T = Ts[p]
                for half in range(2):
                    nc.scalar.activation(
                        feats[1][:, kslice(p, half)], kw_ps[2 * p + half], Sin,
                        scale=om0 / 2, bias=bias_k,
                    )
                nc.scalar.activation(
                    feats[1][:, off_q[p] : off_q[p] + QB],
                    qw_ps[:, 2 * p * Q : (2 * p + 2) * Q], Sin,
                    scale=om0 / 2, bias=bias_q,
                )

            # scaled Chebyshev chain (all-DVE; STT is DVE-only):
            #   feats_2 = (dh + c) * feats_1          (c = 1.5 sin rows / 0.5 cos)
            #   t = (dh + 1) * feats_m ; feats_{m+1} = -0.25*feats_{m-1} + t
            # dh = -2 sin^2(om0 x/2) from the sin-half of feats_1 (the cos-half
            # form cancels catastrophically in fp16).
            sq = consts.tile([64, F], fp16, name="sq")
            dh = consts.tile([128, F], fp16, name="dh")

            def build_dh():
                # k-cols: sin rows are the bottom half; q-cols: the top half
                nc.vector.tensor_tensor(
                    out=sq[:, 0 : off_q[0]], in0=feats[1][64:128, 0 : off_q[0]],
                    in1=feats[1][64:128, 0 : off_q[0]], op=AOp.mult,
                )
                nc.vector.tensor_tensor(
                    out=sq[:, off_q[0] : F], in0=feats[1][0:64, off_q[0] : F],
                    in1=feats[1][0:64, off_q[0] : F], op=AOp.mult,
                )
                nc.vector.tensor_scalar(
                    out=dh[0:64, :], in0=sq, scalar1=-2.0, scalar2=None,
                    op0=AOp.mult,
                )
                nc.vector.tensor_copy(dh[64:128, :], dh[0:64, :])

            def mode_feats(m):
                if m == 2:
                    for c0, c1, pm in (
                        (0, off_q[0], pm15_k),
                        (off_q[0], F, pm15_q),
                    ):
                        nc.vector.scalar_tensor_tensor(
                            out=feats[2][:, c0:c1], in0=dh[:, c0:c1], scalar=pm,
                            in1=feats[1][:, c0:c1], op0=AOp.add, op1=AOp.mult,
                        )
                    return
                t_m = work.tile([128, F], fp16, tag="tmod", bufs=2, name=f"t{m}")
                nc.vector.scalar_tensor_tensor(
                    out=t_m, in0=dh, scalar=1.0, in1=feats[m - 1],
                    op0=AOp.add, op1=AOp.mult,
                )
                nc.vector.scalar_tensor_tensor(
                    out=feats[m], in0=feats[m - 2], scalar=-0.25, in1=t_m,
                    op0=AOp.mult, op1=AOp.add,
                )

            # q-side lhsT: one 4x TS per mode over both pairs' q columns
            lhsT = consts.tile([128, M, 2 * QB], fp16, name="lhsT")

            def lhsT_mode(m):
                nc.vector.tensor_scalar(
                    out=lhsT[:, m - 1, :], in0=feats[m][:, off_q[0] : F],
                    scalar1=wv_col, scalar2=float(bcoef[m - 1] * 4.0 ** (m - 1)),
                    op0=AOp.mult, op1=AOp.mult,
                )

            scores_ps = [
                ps.tile([128, Ts[p]], f32, tag="big", bufs=4,
                        name=f"scores_ps{p}")
                for p in range(NPAIR)
            ]

            def mode_matmuls(m):
                for p in range(NPAIR):
                    for half in range(2):
                        b = 2 * p + half
                        nc.tensor.matmul(
                            scores_ps[p][64 * half : 64 * half + 64, :],
                            lhsT=lhsT[:, m - 1, Q * b : Q * (b + 1)],
                            rhs=feats[m][:, kslice(p, half)],
                            start=(m == 1),
                            stop=(m == M),
                            tile_position=(0, 64 * half),
                        )

            def mask_matmul(p):
                nc.tensor.matmul(
                    scores_ps[p][:, :], lhsT=sel2, rhs=amask[p],
                    start=False, stop=False,
                )

            attn_sbs = [None] * NPAIR
            recips = [None] * NPAIR

            def epi_exp(p):
                T = Ts[p]
                attn_sb = soft.tile([128, T], f32, tag=f"attn{p}", bufs=1,
                                    name=f"attn_sb{p}")
                sumexp = soft.tile([128, 1], f32, tag=f"sumexp{p}", bufs=1,
                                   name=f"sumexp{p}")
                nc.scalar.activation(
                    attn_sb, scores_ps[p][:, 0:T], Exp, accum_out=sumexp
                )
                recip = soft.tile([128, 1], f32, tag=f"recip{p}", bufs=1,
                                  name=f"recip{p}")
                nc.vector.reciprocal(recip, sumexp)
                attn_sbs[p] = attn_sb
                recips[p] = recip

            def epilogue(p):
                T, CK = Ts[p], CKs[p]
                attn_sb = attn_sbs[p]
                recip = recips[p]
                attnT_sb = soft.tile([128, CK, 128], bf16, tag=f"attnT{p}", bufs=1)
                attnT_ps = ps.tile([128, 4, 128], f32, tag="tail", bufs=2)
                for cc in range(CK):
                    cw = min(128, T - 128 * cc)
                    nc.tensor.transpose(
                        attnT_ps[0:cw, cc, :],
                        attn_sb[:, 128 * cc : 128 * cc + cw],
                        ident,
                    )
                for cc in range(CK):
                    cw = min(128, T - 128 * cc)
                    if cc % 2 == 0:
                        nc.vector.tensor_copy(
                            attnT_sb[0:cw, cc, :], attnT_ps[0:cw, cc, :]
                        )
                    else:
                        nc.scalar.activation(
                            attnT_sb[0:cw, cc, :], attnT_ps[0:cw, cc, :], Copy
                        )

                out_pair = soft.tile([128, V], f32, tag=f"out_pair{p}", bufs=1)
                out_ps = ps.tile([128, V], f32, tag="tail", bufs=2)
                for half in range(2):
                    rows = slice(64 * half, 64 * half + 64)
                    for cc in range(CK):
                        cw = min(128, T - 128 * cc)
                        nc.tensor.matmul(
                            out_ps[rows, :],
                            lhsT=attnT_sb[0:cw, cc, rows],
                            rhs=vals_pair[p][0:cw, half, cc, :],
                            start=(cc == 0),
                            stop=(cc == CK - 1),
                            tile_position=(0, 64 * half),
                        )
                nc.vector.tensor_scalar_mul(out=out_pair, in0=out_ps, scalar1=recip)
                nc.sync.dma_start(
                    out=out_d[2 * p : 2 * p + 2].rearrange("b q v -> (b q) v"),
                    in_=out_pair,
                )

            # ---- schedule ----
            fpool_bufs = int(os.environ.get("FBUFS", "5"))
            feats[1] = work.tile([128, F], fp16, tag="feat", bufs=fpool_bufs,
                                 name="feat1")
            mode1_pair(0)
            mode1_pair(1)
            build_dh()
            late_mask(0)
            late_mask(1)
            lhsT_mode(1)
            mode_matmuls(1)
            for m in range(2, M + 1):
                feats[m] = work.tile([128, F], fp16, tag="feat", bufs=fpool_bufs,
                                     name=f"feat{m}")
                mode_feats(m)
                lhsT_mode(m)
                mode_matmuls(m)
                if m == 3:
                    mask_matmul(0)
                    mask_matmul(1)
                    late_vals(0)
                    late_vals(1)
                if m == 4:
                    expdum = consts.tile([1, 1], fp16)
                    nc.scalar.activation(expdum, feats[2][0:1, 0:1], Exp)
            if DBG:
                for i, (nm, t) in enumerate((("f1", feats[1]), ("f2", feats[2]),
                                             ("f5", feats[5]), ("f8", feats[8]),
                                             ("dh", dh))):
                    d32 = consts.tile([128, F], f32, name=f"d32_{nm}")
                    nc.vector.tensor_copy(d32, t)
                    nc.sync.dma_start(out=dbg_d[i, :, 0:F], in_=d32)
                s32 = consts.tile([128, Ts[0]], f32, name="d32_sc")
                nc.vector.tensor_copy(s32, scores_ps[0][:, 0:Ts[0]])
                nc.sync.dma_start(out=dbg_d[5, :, 0:Ts[0]], in_=s32)
            epi_exp(0)
            epi_exp(1)
            epilogue(0)
            epilogue(1)

    nc.compile()
    return nc


_TIME_CACHE = {}


def _class_time(A, Bx):
    key = (A, Bx)
    if key not in _TIME_CACHE:
        try:
            from concourse.timeline_sim import TimelineSim

            best = None
            for km in (True, False):
                nc = build_nc(A, Bx, kdma_merge=km)
                t = float(TimelineSim(nc, trace=False).simulate())
                if best is None or t < best[0]:
                    best = (t, km)
            _TIME_CACHE[key] = best
        except Exception:
            _TIME_CACHE[key] = (20.0 * (A + Bx) + 8000.0, True)
    return _TIME_CACHE[key][0]


def best_cfg(A, Bx):
    _class_time(A, Bx)
    return _TIME_CACHE[(A, Bx)][1:]


def _compositions(n, m):
    if m == 1:
        yield (n,)
        return
    for first in range(1, n - m + 2):
        for rest in _compositions(n - first, m - 1):
            yield (first,) + rest


def plan(vl):
    """Partition 32 batches into 8 cores x (pair0, pair1) and <= MAXPROGS
    program classes (same scheme as the baseline kernel)."""
    vl = np.asarray(vl).reshape(-1).astype(np.int64)
    assert vl.shape[0] == B
    order = np.argsort(-vl, kind="stable")
    pairs = [(int(order[2 * i]), int(order[2 * i + 1])) for i in range(B // 2)]
    ext = [min(K, (int(vl[p[0]]) + 7) // 8 * 8) for p in pairs]

    npair = len(pairs)
    ncore = npair // 2
    slot0 = list(range(ncore))
    slot1 = list(range(ncore, npair))

    cands = []
    for m in range(1, min(MAXPROGS, ncore) + 1):
        for comp in _compositions(ncore, m):
            s0_groups = []
            off = 0
            for nk in comp:
                s0_groups.append(slot0[off : off + nk])
                off += nk
            s1_groups = [None] * m
            off = 0
            for k in reversed(range(m)):
                nk = comp[k]
                s1_groups[k] = slot1[off : off + nk]
                off += nk
            classes = []
            loads = []
            for k in range(m):
                A = max(ext[i] for i in s0_groups[k])
                Bx = max(ext[i] for i in s1_groups[k])
                A, Bx = max(A, Bx), min(A, Bx)
                loads.append(A + Bx)
                classes.append((A, Bx, s0_groups[k], s1_groups[k]))
            cands.append((max(loads), sum(loads), m, classes))
    cands.sort(key=lambda c: c[:3])
    best_load = cands[0][0]
    cands = [c for c in cands if c[0] <= best_load + 64][:24]
    best = None
    for _, _, m, classes in cands:
        t = max(_class_time(A, Bx) for A, Bx, _, _ in classes)
        score = (t, sum(_class_time(A, Bx) for A, Bx, _, _ in classes), m)
        if best is None or score < best[0]:
            best = (score, classes)

    _, classes = best
    out = []
    for A, Bx, a_pairs, b_pairs in classes:
        cores = []
        for i in range(len(a_pairs)):
            pa = pairs[a_pairs[i]]
            pb = pairs[b_pairs[len(b_pairs) - 1 - i]]
            cores.append([pa[0], pa[1], pb[0], pb[1]])
        out.append({"T0": int(A), "T1": int(Bx), "cores": cores})
    return out


def get_nc(T0, T1):
    (km,) = best_cfg(T0, T1)
    key = (T0, T1, km)
    if key not in _NC_CACHE:
        _NC_CACHE[key] = build_nc(T0, T1, kdma_merge=km)
    return _NC_CACHE[key]


def _make_cdata():
    cd = np.zeros((128, 264), dtype=np.float32)
    cd[:, 0:128] = np.eye(128, dtype=np.float32)
    cd[0:64, 128] = np.pi / 2          # bias_k: cos rows on top for k-side
    cd[64:128, 129] = np.pi / 2        # bias_q: cos rows on bottom for q-side
    cd[0:64, 130] = 0.5                # pm15_k
    cd[64:128, 130] = 1.5
    cd[0:64, 131] = 1.5                # pm15_q
    cd[64:128, 131] = 0.5
    cd[0:64, 132] = 7.5 * np.pi        # s2_k: cos rows on top
    cd[64:128, 132] = 7.0 * np.pi
    cd[0:64, 133] = 7.0 * np.pi        # s2_q: sin rows on top
    cd[64:128, 133] = 7.5 * np.pi
    cd[:, 134] = -np.pi                # Sin bias for the mod path
    cd[0, 136:200] = 1.0               # sel2 row0: batch-half 0
    cd[1, 200:264] = 1.0               # sel2 row1: batch-half 1
    return cd


def kernel(queries, keys, values, valid_lens, Wq, Wk, wv):
    from concourse.bass_utils import run_bass_kernel_spmd

    queries = np.ascontiguousarray(queries, dtype=np.float32)
    keys = np.ascontiguousarray(keys, dtype=np.float32)
    values = np.ascontiguousarray(values, dtype=np.float32)
    vl = np.ascontiguousarray(np.asarray(valid_lens).reshape(B), dtype=np.int32)
    Wq = np.ascontiguousarray(Wq, dtype=np.float32)
    Wk = np.ascontiguousarray(Wk, dtype=np.float32)
    wv2 = np.ascontiguousarray(wv, dtype=np.float32).reshape(H, 1)

    out = np.empty((B, Q, V), dtype=np.float32)
    for cls in plan(vl):
        nc = get_nc(cls["T0"], cls["T1"])
        in_maps = []
        for bidx in cls["cores"]:
            in_maps.append(
                {
                    "queries": queries[bidx],
                    "keys": keys[bidx],
                    "values": values[bidx],
                    "valid_lens": vl[bidx].reshape(BPC, 1),
                    "Wq": Wq,
                    "Wk": Wk,
                    "wv": wv2,
                    "cdata": _make_cdata(),
                }
            )
        res = run_bass_kernel_spmd(nc, in_maps, core_ids=list(range(len(in_maps))))
        for bidx, r in zip(cls["cores"], res.results):
            out[bidx] = r["out"]
    return out


if __name__ == "__main__":
    rng = np.random.default_rng(0)
    q = rng.standard_normal((B, Q, H), dtype=np.float32)
    k = rng.standard_normal((B, K, H), dtype=np.float32)
    v = rng.standard_normal((B, K, V), dtype=np.float32)
    vl = rng.integers(1, K + 1, size=(B,)).astype(np.int32)
    Wq = rng.standard_normal((H, H), dtype=np.float32) / np.sqrt(H)
    Wk = rng.standard_normal((H, H), dtype=np.float32) / np.sqrt(H)
    wv = rng.standard_normal((H,), dtype=np.float32) / np.sqrt(H)
    out = kernel(queries=q, keys=k, values=v, valid_lens=vl, Wq=Wq, Wk=Wk, wv=wv)
    print(out.shape, out.dtype, np.abs(out).mean())
